# revision 41
# baseline (speedup 1.0000x reference)
"""Multi-head attention (B=16, N=1024, C=768, H=12) on 8 TRN2 NeuronCores.

Sharding: data-parallel over batch — each core runs the full attention block
for 2 of the 16 batch elements; weights are replicated, no collectives.

Per-core Bass/Tile kernel (bf16 compute, fp32 accumulation):
  phase A: QKV projection. Q^T/K^T output-major ([d, token] head-pair
    tiles, w_qkv^T chunks stationary, each weight streams all 2048 tokens);
    V token-major, packed into vp tiles with a ones column per head
    ([v|1], 65 cols). Input DMAs interleaved in consumption order.
  phase B (per head-pair, batch, n-half): S^T = K_chunk^T @ Q via row-tiled
    K=64 matmuls (the two heads of a pair at tile_position (0,0)/(64,0)
    run concurrently in disjoint PE row groups and fill the two 512-col
    halves of one 2-bank PSUM tile, so one 1024-wide exp on ScalarE serves
    both; 1/sqrt(d) folded into the activation affine; no max subtraction:
    |logits| < ~8, fp32 exp is safe, softmax normalizes the shift away).
    AV with the ones column so the softmax denominators fall out of the
    same matmul (PSUM row 64). Emission is software-pipelined: QK(mc)
    issues one chunk ahead of exp/AV(mc-1) so the strict-FIFO PE queue
    never parks on an AV wait while an independent QK could run.
    Normalization is LAZY and DVE-only at group end (reciprocals of the
    denominator rows into persistent rs tiles + copies of the unnormalized
    O^T into the at tile); the broadcast matmuls and in-place at-scaling
    (B2 jobs) are interleaved two groups later so they hide under matmul
    streaming. An inline recip->broadcast->mul epilogue measured +320us of
    pipeline stalls; lazy+interleaved removes most of it.
  phase C: out^T[o, n] = sum_c wprojT[c, o] aT[c, n] (+bias as a
    per-partition scalar) — each weight tile loads once and streams both
    batches; the host transposes the [768, 2048] result back.

Container-specific findings baked in: (1) this walrus accepts at most
ONE semaphore wait per instruction — excess waits from the Tile scheduler
are hoisted onto injected EventSemaphore instructions in the BIR JSON;
(2) per-matmul LDWEIGHTS is not deduplicated (walrus --enable-ldw-opt=false)
but is hidden by the PE's reorder window, and fp32 matmuls cost 4x stream
cycles so the broadcast matmuls use 16-bit operands; (3) fp16 matmuls
measured ~28% slower than bf16 end-to-end — compute stays bf16; (4) engine
APs need 32-aligned partition bases, so reciprocal rows live at partitions
64/96 of per-group rs tiles (private per group: sharing tile rows between
adjacent groups risks coarse cross-engine deps).

HW phase costs (within-process ablation, (reps5-reps1)/4 differencing):
A ~56us, +QK ~50us (pairs concurrent, ldw hidden), +exp ~160us (ScalarE
is the phase-B floor), +AV ~0-25us (fully overlapped), epilogue was the
dominant cost and is what B2 restructuring attacks. Cross-process timing
noise is +-35%; compare variants within one process (time_variants.py).
"""

import json

import numpy as np
import ml_dtypes
from contextlib import ExitStack

import concourse.bass as bass
import concourse.tile as tile
import concourse.bass2jax as b2j
import concourse.bass_utils as bu
from concourse import mybir
from concourse.bass_utils import run_bass_kernel_spmd

N_CORES = 8

# ---------------------------------------------------------------------------
# walrus single-wait workaround
# ---------------------------------------------------------------------------
_MAX_WAITS = 1
_orig_compile = bu.compile_bir_kernel


def _split_waits(bir_json: bytes) -> bytes:
    d = json.loads(bir_json)
    for f in d.get("functions", []):
        for blk in f.get("blocks", []):
            new_insts = []
            for inst in blk.get("instructions", []):
                si = inst.get("sync_info")
                waits = si.get("on_wait", []) if si else []
                if len(waits) > _MAX_WAITS:
                    extra, keep = waits[:-_MAX_WAITS], waits[-_MAX_WAITS:]
                    for ci in range(0, len(extra), _MAX_WAITS):
                        new_insts.append({
                            "debug": inst.get("debug", 0),
                            "engine": inst["engine"],
                            "ins": [],
                            "name": f"{inst['name']}-wsplit{ci}",
                            "opcode": "EventSemaphore",
                            "outs": [],
                            "sync_info": {
                                "on_update": [],
                                "on_wait": extra[ci:ci + _MAX_WAITS],
                            },
                        })
                    si["on_wait"] = keep
                new_insts.append(inst)
            blk["instructions"] = new_insts
    return json.dumps(d).encode()


def _patched_compile(bir_json, tmpdir, neff_name="file.neff"):
    return _orig_compile(_split_waits(bir_json), tmpdir, neff_name=neff_name)


def _install_patch():
    bu.compile_bir_kernel = _patched_compile
    b2j.compile_bir_kernel = _patched_compile


F32 = mybir.dt.float32
# compute dtype: bf16 (HW peak path; fp16 measured ~40% slower end-to-end)
F16 = mybir.dt.bfloat16

DIM = 768
NH = 12
HD = 64
SCALE = HD ** -0.5
NB = 2
N = 1024
NT = NB * N
NCC = DIM // 128
NHP = NH // 2
SW = 65  # vp slot width per head


def build_attention_nc(reps: int = 1, phases: str = "ABC"):
    nc = bass.Bass("TRN2", target_bir_lowering=False, debug=False)
    xT = nc.declare_dram_parameter("xT", [DIM, NT], F16, isOutput=False)
    wqkvT = nc.declare_dram_parameter("wqkvT", [DIM, 3 * DIM], F16, isOutput=False)
    wprojT = nc.declare_dram_parameter("wprojT", [DIM, DIM], F16, isOutput=False)
    bias = nc.declare_dram_parameter("bias", [DIM, 1], F32, isOutput=False)
    out = nc.declare_dram_parameter("out", [DIM, NT], F32, isOutput=True)

    with tile.TileContext(nc) as tc:
        for rep in range(reps):
            _emit(nc, tc, xT, wqkvT, wprojT, bias, out, rep, phases)
    return nc


def _emit(nc, tc, xT, wqkvT, wprojT, bias, out, rep, phases: str = "ABC"):
    R = f"r{rep}_"
    with ExitStack() as ctx:
        p_const = ctx.enter_context(tc.tile_pool(name=R + "const", bufs=1))
        p_w = ctx.enter_context(tc.tile_pool(name=R + "w", bufs=1))
        p_qk = ctx.enter_context(tc.tile_pool(name=R + "qk", bufs=1))
        p_vp = ctx.enter_context(tc.tile_pool(name=R + "vp", bufs=1))
        p_aT = ctx.enter_context(tc.tile_pool(name=R + "aT", bufs=1))

        # xb/wq chunks feed the first phase-A matmuls: interleave their DMAs
        # in consumption order so compute starts after ~2 transfers, not
        # after the whole input set.
        wq_t, xb = [], []
        for c in range(NCC):
            t = p_w.tile([128, NT], F16, name=R + f"xb{c}")
            nc.sync.dma_start(t[:], xT[c * 128:(c + 1) * 128, :])
            xb.append(t)
            t = p_w.tile([128, 3 * DIM], F16, name=R + f"wq{c}")
            nc.sync.dma_start(t[:], wqkvT[c * 128:(c + 1) * 128, :])
            wq_t.append(t)
        wp_t = []
        for hp in range(NHP):
            t = p_w.tile([128, DIM], F16, name=R + f"wp{hp}")
            nc.sync.dma_start(t[:], wprojT[hp * 128:(hp + 1) * 128, :])
            wp_t.append(t)
        bias_sb = []
        for oc in range(NCC):
            tbs = p_const.tile([128, 1], F32, name=R + f"bias_sb{oc}")
            nc.sync.dma_start(tbs[:], bias[oc * 128:(oc + 1) * 128, :])
            bias_sb.append(tbs)
        ones_sb = p_const.tile([128, 64], F16, name=R + "ones_sb")
        nc.vector.memset(ones_sb[:], 1.0)

        qT_t = [p_qk.tile([128, NT], F16, name=R + f"qT{i}") for i in range(NHP)]
        kT_t = [p_qk.tile([128, NT], F16, name=R + f"kT{i}") for i in range(NHP)]
        vp_t = [p_vp.tile([128, NH * SW], F16, name=R + f"vp{i}")
                for i in range(NT // 128)]
        aT_t = {}

        # ---- phases A+B, interleaved per head-pair ----
        # One [128,1024] PSUM ring (psX) serves the V/QK projections AND the
        # S^T tiles, so phase-B groups for head-pair hp are emitted right
        # after hp's Q/K projection. ScalarE exp work (the phase-B floor)
        # starts ~10us into the kernel instead of after all of phase A,
        # overlapping with the remaining projection matmuls.
        do_B = "B" in phases
        stage = 4
        for ch in "123":
            if "B" + ch in phases:
                stage = int(ch)
        norm_jobs = []  # (g, rs tile, rowA, rowB, at, nh)
        # one rs tile PER group: sharing rows of one tile between adjacent
        # groups risks coarse-grained cross-engine deps (PE reads group g's
        # rows while DVE writes group g+1's) — the ping-pong pathology
        rs_tiles = [p_const.tile([128, 512], F16, name=R + f"rsml{i}")
                    for i in range(NHP * NB * 2)]
        with tc.tile_pool(name=R + "psX", bufs=2, space="PSUM") as p_psX, \
             tc.tile_pool(name=R + "psAcc", bufs=3, space="PSUM") as p_psAcc, \
             tc.tile_pool(name=R + "psB", bufs=1, space="PSUM") as p_psB, \
             tc.tile_pool(name=R + "E", bufs=4) as p_E, \
             tc.tile_pool(name=R + "bc", bufs=3) as p_bc:

            NMC = N // 128
            gidx = 0
            b2_next = 0

            def emit_epilogue(g, pa, at, nh):
                # DVE-only: no PE round trip mid-pipeline (an inline
                # recip->broadcast->mul epilogue measured +320us of stalls)
                rs = rs_tiles[g]
                rowA, rowB = 64, 96
                with nc.allow_low_precision(
                        reason="bf16 reciprocal: 0.4% rel err is within "
                               "the softmax error budget"):
                    nc.vector.reciprocal(rs[rowA:rowA + 1, :],
                                         pa[0][64:65, :])
                    nc.vector.reciprocal(rs[rowB:rowB + 1, :],
                                         pa[1][64:65, :])
                nc.vector.tensor_copy(at[0:64, nh * 512:(nh + 1) * 512],
                                      pa[0][0:64, :])
                nc.vector.tensor_copy(at[64:128, nh * 512:(nh + 1) * 512],
                                      pa[1][0:64, :])
                norm_jobs.append((g, rs, rowA, rowB, at, nh))

            def emit_b2(g, rs, rowA, rowB, at, nh):
                psB = p_psB.tile([128, 512], F32, tag=R + "psB",
                                 name=R + f"psB{g}")
                nc.tensor.matmul(
                    psB[0:64, :], ones_sb[rowA:rowA + 1, 0:64],
                    rs[rowA:rowA + 1, :],
                    start=True, stop=True, tile_position=(rowA, 0),
                )
                nc.tensor.matmul(
                    psB[64:128, :], ones_sb[rowB:rowB + 1, 0:64],
                    rs[rowB:rowB + 1, :],
                    start=True, stop=True, tile_position=(rowB, 64),
                )
                bc = p_bc.tile([128, 512], F16, tag=R + "bc",
                               name=R + f"bc{g}")
                nc.vector.tensor_copy(bc[:], psB[:])
                sl = slice(nh * 512, (nh + 1) * 512)
                nc.vector.tensor_mul(at[0:64, sl], at[0:64, sl], bc[0:64, :])
                nc.vector.tensor_mul(at[64:128, sl], at[64:128, sl],
                                     bc[64:128, :])

            # V token-major first: every AV below reads the vp tiles
            for nn in range(NT // 128):
                psv = p_psX.tile([128, 1024], F32, tag=R + "psX",
                                 name=R + f"psv{nn}")
                for c in range(NCC):
                    nc.tensor.matmul(
                        psv[:, 0:512],
                        xb[c][:, nn * 128:(nn + 1) * 128],
                        wq_t[c][:, 2 * DIM:2 * DIM + 512],
                        start=(c == 0), stop=(c == NCC - 1),
                    )
                    nc.tensor.matmul(
                        psv[:, 512:768],
                        xb[c][:, nn * 128:(nn + 1) * 128],
                        wq_t[c][:, 2 * DIM + 512:3 * DIM],
                        start=(c == 0), stop=(c == NCC - 1),
                    )
                vp = vp_t[nn]
                nc.vector.memset(vp[:], 1.0)
                nc.vector.tensor_copy(
                    vp[:].rearrange("p (h e) -> p h e", e=SW)[:, :, 0:HD],
                    psv[:, 0:768].rearrange("p (h d) -> p h d", d=HD),
                )

            def emit_qkproj(hp):
                for which, dst in ((hp, qT_t[hp]), (NHP + hp, kT_t[hp])):
                    psq = [p_psX.tile([128, 1024], F32, tag=R + "psX",
                                      name=R + f"psq{which}_{half}")
                           for half in range(2)]
                    for c in range(NCC):
                        for half in range(2):
                            for qh in range(2):
                                nc.tensor.matmul(
                                    psq[half][:, qh * 512:(qh + 1) * 512],
                                    wq_t[c][:, which * 128:(which + 1) * 128],
                                    xb[c][:, half * 1024 + qh * 512:
                                          half * 1024 + (qh + 1) * 512],
                                    start=(c == 0), stop=(c == NCC - 1),
                                )
                    for half in range(2):
                        nc.vector.tensor_copy(
                            dst[:, half * 1024:(half + 1) * 1024], psq[half][:])

            for hp in range(NHP):
                emit_qkproj(hp)
                if not do_B:
                    continue
                for b in range(NB):
                    boff = b * N
                    at = p_aT.tile([128, N], F16, name=R + f"aT{b}_{hp}")
                    for nh in range(2):
                        noff = boff + nh * 512
                        tag = f"{b}_{hp}_{nh}"
                        pa = [p_psAcc.tile([128, 512], F32, tag=R + "psAcc",
                                           name=R + f"pa{tag}_{i}")
                              for i in range(2)] if stage >= 3 else None
                        ps_t = [None] * NMC

                        def emit_qk(mc, nh=nh, noff=noff, boff=boff, tag=tag,
                                    ps_t=ps_t, hp=hp):
                            ps = p_psX.tile([128, 1024], F32, tag=R + "psX",
                                            name=R + f"psS{tag}_{mc}")
                            ps_t[mc] = ps
                            for hi in range(2):
                                pb_ = hi * 64
                                nc.tensor.matmul(
                                    ps[:, hi * 512:(hi + 1) * 512],
                                    kT_t[hp][pb_:pb_ + 64,
                                             boff + mc * 128:
                                             boff + (mc + 1) * 128],
                                    qT_t[hp][pb_:pb_ + 64, noff:noff + 512],
                                    start=True, stop=True,
                                    tile_position=(pb_, 0),
                                )

                        def emit_exp_av(mc, boff=boff, tag=tag, pa=pa,
                                        ps_t=ps_t, hp=hp):
                            if stage < 2:
                                return
                            e = p_E.tile([128, 1024], F16, tag=R + "E",
                                         name=R + f"e{tag}_{mc}")
                            nc.scalar.activation(
                                e[:], ps_t[mc][:],
                                mybir.ActivationFunctionType.Exp,
                                scale=SCALE,
                            )
                            if stage < 3:
                                return
                            vslot = vp_t[(boff + mc * 128) // 128]
                            for hi in range(2):
                                h = 2 * hp + hi
                                nc.tensor.matmul(
                                    pa[hi][0:65, :],
                                    vslot[:, h * SW:h * SW + SW],
                                    e[:, hi * 512:(hi + 1) * 512],
                                    start=(mc == 0), stop=(mc == NMC - 1),
                                )

                        for mc in range(NMC):
                            emit_qk(mc)
                            if mc == 2 and stage >= 4:
                                # interleave deferred B2 normalization jobs
                                # (two groups back) so their PE/DVE work
                                # hides under matmul streaming
                                while (b2_next <= gidx - 2
                                       and b2_next < len(norm_jobs)):
                                    emit_b2(*norm_jobs[b2_next])
                                    b2_next += 1
                            if mc >= 1:
                                emit_exp_av(mc - 1)
                        emit_exp_av(NMC - 1)
                        if stage >= 4:
                            emit_epilogue(gidx, pa, at, nh)
                        gidx += 1
                    aT_t[(b, hp)] = at

            # drain the B2 normalization jobs of the last two groups
            while b2_next < len(norm_jobs):
                emit_b2(*norm_jobs[b2_next])
                b2_next += 1

        if "C" not in phases:
            return
        # ---- phase C: transposed output projection ----
        # out^T[o, n] = sum_c wprojT[c, o] * aT[c, n]; weights (c,o) loaded
        # once and stream both batches (4 matmuls per load).
        with tc.tile_pool(name=R + "psP", bufs=4, space="PSUM") as p_psP, \
             tc.tile_pool(name=R + "ob", bufs=4) as p_ob:
            for oc in range(NCC):
                pp = [p_psP.tile([128, N], F32, tag=R + "psP",
                                 name=R + f"pp{oc}_{b}") for b in range(NB)]
                for cp in range(NHP):
                    lhs = wp_t[cp][:, oc * 128:(oc + 1) * 128]
                    for b in range(NB):
                        for nh in range(2):
                            nc.tensor.matmul(
                                pp[b][:, nh * 512:(nh + 1) * 512],
                                lhs,
                                aT_t[(b, cp)][:, nh * 512:(nh + 1) * 512],
                                start=(cp == 0), stop=(cp == NHP - 1))
                for b in range(NB):
                    ob = p_ob.tile([128, N], F32, tag=R + "ob")
                    nc.vector.tensor_scalar_add(ob[:], pp[b][:], bias_sb[oc][:])
                    nc.sync.dma_start(
                        out[oc * 128:(oc + 1) * 128, b * N:(b + 1) * N], ob[:])


# ---------------------------------------------------------------------------
# host wrapper
# ---------------------------------------------------------------------------
_CACHE = {}


def _prep_in_maps(x, w_qkv, w_proj, b_proj):
    x = np.asarray(x, dtype=np.float32)
    wqkvT = np.ascontiguousarray(np.asarray(w_qkv, dtype=np.float32).T
                                 ).astype(ml_dtypes.bfloat16)
    wprojT = np.ascontiguousarray(np.asarray(w_proj, dtype=np.float32).T
                                  ).astype(ml_dtypes.bfloat16)
    bias = np.asarray(b_proj, dtype=np.float32).reshape(DIM, 1).copy()
    in_maps = []
    for c in range(N_CORES):
        xs = x[c * NB:(c + 1) * NB]                       # [2, 1024, 768]
        xT = np.ascontiguousarray(xs.transpose(2, 0, 1).reshape(DIM, NT))
        in_maps.append({
            "xT": xT.astype(ml_dtypes.bfloat16),
            "wqkvT": wqkvT,
            "wprojT": wprojT,
            "bias": bias,
        })
    return in_maps


def kernel(x, w_qkv, w_proj, b_proj):
    _install_patch()
    if "nc" not in _CACHE:
        _CACHE["nc"] = build_attention_nc(1)
    nc = _CACHE["nc"]
    in_maps = _prep_in_maps(x, w_qkv, w_proj, b_proj)
    res = run_bass_kernel_spmd(nc, in_maps, core_ids=list(range(N_CORES)))
    shards = []
    for c in range(N_CORES):
        oT = res.results[c]["out"]                        # [768, 2048]
        shards.append(oT.T.reshape(NB, N, DIM))
    return np.ascontiguousarray(
        np.concatenate(shards, axis=0)).astype(np.float32)
